# revision 38
# baseline (speedup 1.0000x reference)
"""Trainium2 Bass kernel: segment-mean over token segments + pairwise-diff edge MLP.

Reference computation (per batch row b):
  seg = cumsum(ids == 3); valid = ids != 3
  means[n] = mean of features[s] over tokens with seg==n & valid (n < 8), 0-count -> 0
  diff[i,j] = means[i] - means[j]                               # [8,8,H]
  out[i,j]  = relu(relu(diff @ W1 + b1) @ Wm + bm) @ W2 + b2    # [8,8,150]

Key structural ideas:
  1. Only tokens BEFORE the 8th separator contribute (seg < 8) -- ~6% of the
     sequence for uniform ids. The host gathers just the valid tokens into a
     single dense per-core stream (it already derives the one-hot from ids)
     and balances rows across cores by token count, so the device streams
     ~1.8 MB instead of 50 MB per core.
  2. diff is linear: relu((m_i - m_j) @ W1 + b1) == relu(u_i - u_j + b1)
     with u = m @ W1. Projecting the 128 means (16 rows x 8 segs) through W1
     first shrinks the big matmul's moving data 8x vs projecting all 1024
     pairwise diffs.
  3. All 16 rows share one (row,seg)=128 partition space: stage-1 uses a
     block-diagonal one-hot stationary so a single accumulation chain + one
     transpose + one W1 projection serves the whole core (no per-row loops).
  4. b2 is folded into an augmented W2 row driven by a constant ones-row in
     the h2 tail tile; 1/count is an exact fp32 per-partition activation
     scale at stage-1 eviction; b1/bm ride the relu evictions.

Distribution: data-parallel over batch B=128 across 8 NeuronCores (16 rows
per core). All matmul operands bf16 (fp32 PSUM accumulate); output fp32.

Device pipeline per core:
  s1:   meansAll[(r,seg), h]: per 128-token chunk, stationary = block-diag
        one-hot [128t, 128(r,seg)], moving = features [128t, 384]x2 halves,
        accumulated over chunks; evict with 1/count scale -> ms bf16.
  tr:   meansT[h, (r,seg)] via 6 PE transposes (identity moving).
  u:    u[(r,seg), c] = meansT^T @ W1 (6 accumulating matmuls, 150 cols).
  diff: h1T[c, (r,i,j)] = relu(u^T @ E16 + b1); E16 = +-1 block-diag pair
        matrix [128, 896] (diagonal pairs skipped -- their output is a
        feature-independent constant the host computes exactly in fp32);
        2 c-chunks x 2 pair-halves (512+384).
  mm2:  h2T[c', pairs] = relu(Wm^T @ h1T + bm); 2 k-chunks x 2 c' x 2 halves.
  mm3:  out[pair, cc] = h2T^T @ W2aug (7 pair-chunks; ones row adds b2).
"""

import math
import sys

import numpy as np

if "/opt/trn_rl_repo" not in sys.path:
    sys.path.insert(0, "/opt/trn_rl_repo")

import ml_dtypes

import concourse.bass as bass
import concourse.mybir as mybir
from concourse.bass import ds
from concourse.bass_utils import run_bass_kernel_spmd
from concourse.tile import TileContext

B, S, H, C = 128, 1024, 768, 150
NSEG = 8
SEP_ID = 3
NCORES = 8
RPC = 16                    # rows per core
NPR = RPC * NSEG * (NSEG - 1)  # 896 off-diagonal pair-columns per core
HFS = (512, 384)            # pair-half split (aligned to 128 for mm3 chunks)
HC = H // 128               # 6 hidden chunks

F32 = mybir.dt.float32
BF16 = mybir.dt.bfloat16
NPBF = ml_dtypes.bfloat16

# cw (shared bf16 const) column layout; [0, CWA) is needed first (the W1
# projection) and arrives in an earlier DMA than the rest. The transpose
# identity is generated on-device (iota + is_equal) instead of DMA'd.
W1OFF = 0                      # [128, 6*150]  W1p[h, hc*150+c] = W1[hc*128+h, c]
CWA = W1OFF + HC * C
WM0OFF = CWA                   # [128, 150]    Wm[0:128, :]
WM1OFF = WM0OFF + C            # [22, 150]     Wm[128:150, :]
W2AOFF = WM1OFF + C            # [128, 150]    W2[0:128, :]
W2BOFF = W2AOFF + C            # [23, 150]     rows 0..21 = W2[128:150,:], row 22 = b2
E16OFF = W2BOFF + C            # [128, 896]    E16[(r,seg),(r',i,j)], j != i
CW = E16OFF + NPR

# fsc (per-core fp32 const) columns: b1[0:128] | b1[128:150] | bm[0:128] |
# bm[128:150] | 1/count[(r,seg)]
ADD = mybir.AluOpType.add
MAX = mybir.AluOpType.max


def build_program(chg, lastw):
    """chg = number of 128-token chunks in the core's gathered valid stream;
    lastw = used token slots in the final chunk (its DMA and matmuls are
    partition-sliced to skip the padding)."""
    nc = bass.Bass("TRN2", target_bir_lowering=False, debug=False)

    CHW = H + 128   # per-chunk stream: 768 feature cols + 128 one-hot cols
    featg_d = nc.dram_tensor("featg", [128, chg * CHW], BF16,
                             kind="ExternalInput").ap()
    cw_d = nc.dram_tensor("cw", [128, CW], BF16, kind="ExternalInput").ap()
    fsc_d = nc.dram_tensor("fsc", [128, 5], F32, kind="ExternalInput").ap()
    out_d = nc.dram_tensor("out", [NPR, C], BF16,
                           kind="ExternalOutput").ap()

    RELU = mybir.ActivationFunctionType.Relu
    COPY = mybir.ActivationFunctionType.Copy

    # feature DMA granularity: chunk pairs, alternating queues
    dch = 2
    nfd = math.ceil(chg / dch)

    with TileContext(nc) as tc:
        with (
            tc.tile_pool(name="const", bufs=1) as constp,
            tc.tile_pool(name="featp", bufs=max(2, nfd)) as featp,
            tc.tile_pool(name="msp", bufs=1) as msp,
            tc.tile_pool(name="mtp", bufs=1) as mtp,
            tc.tile_pool(name="upl", bufs=1) as upl,
            tc.tile_pool(name="h1p", bufs=1) as h1p,
            tc.tile_pool(name="obp", bufs=4) as obp,
            tc.tile_pool(name="s1p", bufs=2, space="PSUM") as s1p,
            tc.tile_pool(name="tpp", bufs=1, space="PSUM") as tpp,
            tc.tile_pool(name="ups", bufs=1, space="PSUM") as ups,
            tc.tile_pool(name="mp", bufs=4, space="PSUM") as mp,
        ):
            # ---- input DMAs. The one-hot rides inside each feature chunk
            # (cols 768:896) so stage-1's stationary lands with its chunk and
            # no const DMA precedes the feature stream. Consts needed after
            # stage 1 trail the feature chunks. ----
            feat_sb = []
            for fd in range(nfd):
                w = min(dch * CHW, chg * CHW - fd * dch * CHW)
                pw = lastw if fd == nfd - 1 else 128
                t = featp.tile([128, dch * CHW], BF16, tag="feat",
                               name=f"feat{fd}")
                eng = nc.sync if fd % 2 == 0 else nc.scalar
                eng.dma_start(out=t[0:pw, 0:w],
                              in_=featg_d[0:pw, ds(fd * dch * CHW, w)])
                feat_sb.append(t)
            fsc_sb = constp.tile([128, 5], F32, tag="c_fsc")
            nc.scalar.dma_start(out=fsc_sb, in_=fsc_d)
            cw_sb = constp.tile([128, CW], BF16, tag="c_cw")
            nc.scalar.dma_start(out=cw_sb[:, 0:CWA], in_=cw_d[:, 0:CWA])
            nc.scalar.dma_start(out=cw_sb[:, CWA:CW],
                                in_=cw_d[:, ds(CWA, CW - CWA)])
            # h2b rows 0..21 = h2T tail (runtime), row 22 = const ones (b2 row)
            h2b_sb = constp.tile([23, NPR], BF16, tag="c_h2b")
            nc.gpsimd.memset(h2b_sb, 1.0)
            # transpose identity built on the idle vector engine: idx = i - p,
            # then (idx == 0) -> 1.0
            idx_sb = constp.tile([128, 128], mybir.dt.int16, tag="c_idx")
            nc.gpsimd.iota(idx_sb, [[1, 128]], base=0, channel_multiplier=-1)
            i128_sb = constp.tile([128, 128], BF16, tag="c_i128")
            nc.gpsimd.tensor_scalar(i128_sb, idx_sb, 0, None,
                                    mybir.AluOpType.is_equal)

            # ---- stage 1: meansAll [(r,seg), h] ----
            # chunk-major order keeps PE consumption rate matched to DMA
            # arrival; the final chunk is half-split so half0's eviction
            # hides behind the last half1 matmul
            s1 = [s1p.tile([128, 384], F32, tag="s1", name=f"s1_{i}")
                  for i in range(2)]
            ms = msp.tile([128, H], BF16, tag="ms")
            tp = tpp.tile([128, H], BF16, tag="tp")
            mt = mtp.tile([128, H], BF16, tag="mt")
            up_ = ups.tile([128, C], F32, tag="up")
            for ch in range(chg):
                ft = feat_sb[ch // dch]
                fo = (ch % dch) * CHW
                pw = lastw if ch == chg - 1 else 128
                for half in range(2):
                    nc.tensor.matmul(
                        s1[half],
                        ft[0:pw, ds(fo + H, 128)],
                        ft[0:pw, ds(fo + half * 384, 384)],
                        start=(ch == 0),
                        stop=(ch == chg - 1),
                    )
                    if ch == chg - 1 and half == 0:
                        nc.scalar.activation(
                            ms[:, 0:384], s1[0], COPY, scale=fsc_sb[:, 4:5],
                        )
            nc.scalar.activation(
                ms[:, 384:768], s1[1], COPY, scale=fsc_sb[:, 4:5],
            )
            for half in range(2):
                for hc3 in range(3):
                    hc = half * 3 + hc3
                    nc.tensor.transpose(
                        tp[:, ds(hc * 128, 128)],
                        ms[:, ds(hc * 128, 128)],
                        i128_sb,
                    )
                nc.vector.tensor_copy(mt[:, ds(half * 384, 384)],
                                      tp[:, ds(half * 384, 384)])
            for hc in range(HC):
                nc.tensor.matmul(
                    up_,
                    mt[:, ds(hc * 128, 128)],
                    cw_sb[:, ds(W1OFF + hc * C, C)],
                    start=(hc == 0),
                    stop=(hc == HC - 1),
                )
            u = upl.tile([128, C], BF16, tag="u")
            nc.vector.tensor_copy(u, up_)

            # ---- diff + relu: h1T [c, pairs], halves interleaved so the
            # h0 evictions overlap the h1 matmuls ----
            h1a = h1p.tile([128, NPR], BF16, tag="h1a")
            h1b = h1p.tile([22, NPR], BF16, tag="h1b")
            ho = (0, HFS[0])
            dps = {}
            for hf in range(2):      # pair half
                for cc in range(2):  # c chunk: 0 -> [0:128], 1 -> [128:150]
                    csz = 128 if cc == 0 else 22
                    p = mp.tile([128, 512], F32, tag="m", name=f"d_{cc}_{hf}")
                    nc.tensor.matmul(
                        p[0:csz, 0:HFS[hf]],
                        u[:, ds(cc * 128, csz)],
                        cw_sb[:, ds(E16OFF + ho[hf], HFS[hf])],
                        start=True, stop=True,
                    )
                    dps[(cc, hf)] = p
                nc.scalar.activation(h1a[:, ds(ho[hf], HFS[hf])],
                                     dps[(0, hf)][0:128, 0:HFS[hf]], RELU,
                                     bias=fsc_sb[0:128, 0:1])
                nc.vector.tensor_scalar(h1b[:, ds(ho[hf], HFS[hf])],
                                        dps[(1, hf)][0:22, 0:HFS[hf]],
                                        fsc_sb[0:22, 1:2], 0.0, ADD, MAX)

            # ---- mm2: h2T [c', pairs] ----
            h2a = h1p.tile([128, NPR], BF16, tag="h2a")
            for hf in range(2):
                eps = {}
                for cc in range(2):
                    csz = 128 if cc == 0 else 22
                    p = mp.tile([128, 512], F32, tag="m", name=f"e_{cc}_{hf}")
                    nc.tensor.matmul(p[0:csz, 0:HFS[hf]],
                                     cw_sb[0:128, ds(WM0OFF + cc * 128, csz)],
                                     h1a[:, ds(ho[hf], HFS[hf])],
                                     start=True, stop=False)
                    nc.tensor.matmul(p[0:csz, 0:HFS[hf]],
                                     cw_sb[0:22, ds(WM1OFF + cc * 128, csz)],
                                     h1b[:, ds(ho[hf], HFS[hf])],
                                     start=False, stop=True)
                    eps[cc] = p
                nc.scalar.activation(h2a[:, ds(ho[hf], HFS[hf])],
                                     eps[0][0:128, 0:HFS[hf]], RELU,
                                     bias=fsc_sb[0:128, 2:3])
                nc.vector.tensor_scalar(h2b_sb[0:22, ds(ho[hf], HFS[hf])],
                                        eps[1][0:22, 0:HFS[hf]],
                                        fsc_sb[0:22, 3:4], 0.0, ADD, MAX)

            # ---- mm3: out [pair, cc] (last DMA kept small for the tail) ----
            for grp in range(4):
                npc = 2 if grp < 3 else 1
                ob = obp.tile([128, 2, C], BF16, tag="ob", name=f"ob{grp}")
                for pc2 in range(npc):
                    pc = grp * 2 + pc2
                    op_ = mp.tile([128, 512], F32, tag="m",
                                  name=f"op_{pc}")[:, 0:C]
                    nc.tensor.matmul(op_, h2a[:, ds(pc * 128, 128)],
                                     cw_sb[0:128, ds(W2AOFF, C)],
                                     start=True, stop=False)
                    nc.tensor.matmul(op_, h2b_sb[:, ds(pc * 128, 128)],
                                     cw_sb[0:23, ds(W2BOFF, C)],
                                     start=False, stop=True)
                    eng_copy = (nc.scalar.copy if pc2 % 2 == 0
                                else nc.vector.tensor_copy)
                    eng_copy(ob[:, pc2, :], op_)
                nc.sync.dma_start(
                    out=out_d[ds(grp * 256, npc * 128), :].rearrange(
                        "(t p) c -> p t c", p=128),
                    in_=ob[:, 0:npc, :],
                )

    import bass_rust as _bass_rust
    _bass_rust.move_matmul_waits_to_ldweights(nc.m)
    _bass_rust.generate_event_semaphores(nc)
    return nc


def host_prep(output_ids, features, W1, b1, Wm, bm, W2, b2):
    ids = np.asarray(output_ids)
    feats = np.asarray(features, dtype=np.float32)
    nrows = ids.shape[0]
    ncores = nrows // RPC

    is_sep = ids == SEP_ID
    seg = np.cumsum(is_sep.astype(np.int64), axis=1)
    valid = (~is_sep) & (seg < NSEG)
    L = valid.sum(axis=1)

    counts = np.zeros((nrows, NSEG), np.int64)
    bb, tt = np.nonzero(valid)
    np.add.at(counts, (bb, seg[bb, tt]), 1)
    inv = (1.0 / np.maximum(counts, 1)).astype(np.float32)

    # balance rows across cores by valid-token count (greedy, inverted at
    # gather time) so the slowest core carries ~1/ncores of the total load
    order = np.argsort(-L, kind="stable")
    perm = np.full((ncores, RPC), -1, np.int64)
    loads = np.zeros(ncores, np.int64)
    nas = np.zeros(ncores, np.int64)
    for row in order:
        cands = [c for c in range(ncores) if nas[c] < RPC]
        c = min(cands, key=lambda x: loads[x])
        perm[c, nas[c]] = row
        nas[c] += 1
        loads[c] += L[row]
    chg = max(1, math.ceil(int(loads.max()) / 128))
    assert chg <= 48, f"core token load too large: {loads.max()}"
    lastw = 128

    featg = np.zeros((ncores, 128, chg, H + 128), NPBF)
    fsc = np.zeros((ncores, 128, 5), np.float32)
    b1 = np.asarray(b1, np.float32)
    bm = np.asarray(bm, np.float32)
    for c in range(ncores):
        fsc[c, 0:128, 0] = b1[0:128]
        fsc[c, 0:22, 1] = b1[128:150]
        fsc[c, 0:128, 2] = bm[0:128]
        fsc[c, 0:22, 3] = bm[128:150]
        cursor = 0
        for r in range(RPC):
            row = perm[c, r]
            fsc[c, r * NSEG:(r + 1) * NSEG, 4] = inv[row]
            toks = np.nonzero(valid[row])[0]
            n = len(toks)
            slot = cursor + np.arange(n)
            p, ch = slot % 128, slot // 128
            featg[c, p, ch, 0:H] = feats[row, toks, :].astype(NPBF)
            featg[c, p, ch, H + r * NSEG + seg[row, toks]] = 1.0
            cursor += n

    # shared bf16 constants
    W1 = np.asarray(W1, np.float32)
    Wm = np.asarray(Wm, np.float32)
    W2 = np.asarray(W2, np.float32)
    b2 = np.asarray(b2, np.float32)
    cw = np.zeros((128, CW), NPBF)
    cw[:, W1OFF:W1OFF + HC * C] = (
        W1.reshape(HC, 128, C).transpose(1, 0, 2).reshape(128, HC * C)
        .astype(NPBF))
    e16 = np.zeros((128, NPR), np.float32)
    eye = np.eye(NSEG, dtype=np.float32)
    base = eye[:, :, None] - eye[:, None, :]          # [seg, i, j]
    offd = np.stack([base[:, i, [j for j in range(NSEG) if j != i]]
                     for i in range(NSEG)], axis=1)   # [seg, i, 7]
    for r in range(RPC):
        e16[r * NSEG:(r + 1) * NSEG, r * 56:(r + 1) * 56] = (
            offd.reshape(NSEG, 56))
    cw[:, E16OFF:E16OFF + NPR] = e16.astype(NPBF)
    cw[0:128, WM0OFF:WM0OFF + C] = Wm[0:128].astype(NPBF)
    cw[0:22, WM1OFF:WM1OFF + C] = Wm[128:150].astype(NPBF)
    cw[0:128, W2AOFF:W2AOFF + C] = W2[0:128].astype(NPBF)
    cw[0:22, W2BOFF:W2BOFF + C] = W2[128:150].astype(NPBF)
    cw[22, W2BOFF:W2BOFF + C] = b2.astype(NPBF)

    in_maps = []
    for c in range(ncores):
        in_maps.append(dict(
            featg=np.ascontiguousarray(featg[c].reshape(128, chg * (H + 128))),
            cw=cw, fsc=fsc[c],
        ))
    return in_maps, chg, lastw, perm


def gather_output(core_outs, diag, perm):
    ncores = len(core_outs)
    full = np.empty((NSEG, NSEG, ncores * RPC, C), np.float32)
    offj = np.array([[j for j in range(NSEG) if j != i] for i in range(NSEG)])
    for c, o in enumerate(core_outs):
        o = o.astype(np.float32).reshape(RPC, NSEG, NSEG - 1, C)
        for i in range(NSEG):
            full[i, offj[i][:, None], perm[c], :] = (
                o[:, i, :, :].transpose(1, 0, 2))
    full[np.arange(NSEG), np.arange(NSEG)] = diag[None, None, :]
    return full


def diag_constant(W1, b1, Wm, bm, W2, b2):
    """MLP output for diff == 0 (diagonal pairs), exact in fp32."""
    h1 = np.maximum(np.asarray(b1, np.float32), 0)
    h2 = np.maximum(h1 @ np.asarray(Wm, np.float32)
                    + np.asarray(bm, np.float32), 0)
    return h2 @ np.asarray(W2, np.float32) + np.asarray(b2, np.float32)


_NC_CACHE = {}


def _get_program(chg, lastw):
    key = (chg, lastw)
    if key not in _NC_CACHE:
        _NC_CACHE[key] = build_program(chg, lastw)
    return _NC_CACHE[key]


def run(inputs, trace=False, trace_cores=None):
    in_maps, chg, lastw, perm = host_prep(**inputs)
    nc = _get_program(chg, lastw)
    res = run_bass_kernel_spmd(
        nc, in_maps, core_ids=list(range(NCORES)),
        trace=trace, trace_cores=trace_cores,
    )
    diag = diag_constant(inputs["W1"], inputs["b1"], inputs["Wm"],
                         inputs["bm"], inputs["W2"], inputs["b2"])
    out = gather_output([r["out"] for r in res.results], diag, perm)
    return out, res


def kernel(**inputs):
    out, _ = run(inputs, trace=False)
    return out
